# revision 7
# baseline (speedup 1.0000x reference)
"""Multi-head graph attention kernel for Trainium2 (8 NeuronCores, SPMD).

Math (algebraically equivalent to the reference):
  ew_e   = sigmoid(sum(edge_attr[e]))
  a_e    = ew_e * SCALE / max(deg[dst_e], 1)
  Gx[n]  = sum_{e: dst=n} a_e * x[src_e]            (segment sum of gathered rows)
  G      = Gx @ w_q ;  K = x @ w_k ;  V = x @ w_v
  S[n,h] = sum_{d in head h} K[n,d] * G[n,d]
  out    = (V * repeat(S, 16)) @ w_o + b_o

Device-side design (v2 — transposed, bf16, host-built masks):
  * Everything on device is bf16 (fp32 PSUM accumulation), halving both the
    random-row gather traffic and the streamed operands.
  * Nodes are permuted into NCORES*NW windows of 128 slots; every edge lives
    with its destination's window.  Edges are grouped by src range
    (src < 32768 vs >= 32768, so dma_gather's int16 indices can address the
    bf16 row directly) and padded to CA/CB chunks of 128.
  * The per-edge scatter coefficients a_e are baked on the HOST into a dense
    bf16 "mask" M[e_pos, slot] per chunk (one nonzero per row), streamed
    sequentially via HWDGE.  This removes the per-chunk DVE one-hot build —
    the v1 bottleneck (Vector 94.6% busy from SBUF-port contention with the
    SWDGE gather descriptor rings).
  * All compute is done transposed (features on partitions): the chunk
    matmul accumulates G^T[d, slot] = blk^T @ mask with the gathered rows as
    the stationary operand, so the epilogue needs zero PE transposes:
        K^T = wk^T X^T,  V^T = wv^T X^T,  Ghat^T = wq^T G^T
        S = Hmat (K^T*Ghat^T)   (per-head partition-group reduce via PE)
        out^T = wo^T (V^T * Hmat^T S) + b
    The epilogue is batched over NBATCH=4 windows ([128, 512] tiles).
"""

import math
import numpy as np

# ---------------- problem constants (hardcoded per the task) ----------------
N = 50000
E = 800000
D = 128
H = 8
DH = 16
DE = 16
SCALE = 1.0 / math.sqrt(DH)
NCORES = 8
P = 128          # node slots per window / partition dim
NW = 49          # windows per core  (NCORES*NW*P = 50176 >= N)
NBATCH = 4       # windows per epilogue batch
GMAX = 4         # max chunks (512 idx) per dma_gather call; small calls let
                 # the 96-desc/queue SWDGE rings hold 2+ calls so descriptor
                 # generation overlaps the previous call's drain
SPLIT = 32768    # src-range split so gather indices fit int16


def _call_sizes(C):
    """Split C chunks into dma_gather call sizes of at most GMAX chunks."""
    out = []
    while C > 0:
        out.append(min(GMAX, C))
        C -= out[-1]
    return out


# ======================= host-side preprocessing ===========================

def preprocess(edge_index):
    """Index-only preprocessing: node permutation, edge grouping, padding."""
    src = np.asarray(edge_index[0], dtype=np.int64)
    dst = np.asarray(edge_index[1], dtype=np.int64)

    deg = np.bincount(dst, minlength=N)

    # node -> (window, slot): snake-deal by degree for load balance
    nwin_total = NCORES * NW
    order = np.argsort(-deg, kind="stable")
    slot_of_node = np.empty(N, dtype=np.int64)
    win_of_node = np.empty(N, dtype=np.int64)
    for r in range((N + nwin_total - 1) // nwin_total):
        chunk = order[r * nwin_total:(r + 1) * nwin_total]
        wins = np.arange(len(chunk))
        if r % 2 == 1:
            wins = nwin_total - 1 - wins
        win_of_node[chunk] = wins
        slot_of_node[chunk] = r
    assert slot_of_node.max() < P

    perm = np.full(nwin_total * P, -1, dtype=np.int64)
    perm[win_of_node * P + slot_of_node] = np.arange(N)

    # edges -> (window, src-range group), sorted by src inside each group.
    # Duplicate srcs within a group are DEDUPED: they share one gather slot
    # and the mask row gets multiple nonzeros.
    e_win = win_of_node[dst]
    e_grp = (src >= SPLIT).astype(np.int64)
    e_key = e_win * 2 + e_grp
    e_order = np.lexsort((src, e_key))
    g_src = src[e_order]
    g_dst = dst[e_order]

    ngrp = nwin_total * 2
    counts = np.bincount(e_key[e_order], minlength=ngrp)
    grp_start = np.concatenate([[0], np.cumsum(counts)])

    # unique-src slot within each group
    new_grp = np.zeros(E, dtype=bool)
    new_grp[grp_start[:-1][counts > 0]] = True
    new_src = np.empty(E, dtype=bool)
    new_src[0] = True
    new_src[1:] = g_src[1:] != g_src[:-1]
    is_slot = new_grp | new_src
    slot_within = np.arange(E) - np.maximum.accumulate(
        np.where(new_grp, np.arange(E), -1))
    # slot_within counts edges; recompute as unique-count within group:
    cum_slots = np.cumsum(is_slot)
    grp_of_edge = np.repeat(np.arange(ngrp), counts)
    slots_before_grp = np.concatenate(
        [[0], cum_slots[grp_start[1:-1] - 1]]) if ngrp > 1 else np.array([0])
    slot_within = cum_slots - 1 - slots_before_grp[grp_of_edge]
    ucounts = np.bincount(grp_of_edge[is_slot], minlength=ngrp)

    CA = int(np.ceil(ucounts[0::2].max() / P))
    CB = int(np.ceil(ucounts[1::2].max() / P))

    batches = [list(range(b, min(b + NBATCH, NW))) for b in range(0, NW, NBATCH)]
    SLOTS_W = (CA + CB) * P
    SLOTS_CORE = NW * SLOTS_W

    # slot layout: per batch, [A-chunks of its windows][B-chunks of its windows]
    batch_base = {}
    base = 0
    for b, wins in enumerate(batches):
        batch_base[b] = base
        base += len(wins) * SLOTS_W

    def group_slot_offset(w, grp):
        b, i = w // NBATCH, w % NBATCH
        nb = len(batches[b])
        if grp == 0:
            return batch_base[b] + i * CA * P
        return batch_base[b] + nb * CA * P + i * CB * P

    goff = np.zeros(ngrp, dtype=np.int64)
    for gw in range(nwin_total):
        w = gw % NW
        goff[gw * 2] = group_slot_offset(w, 0)
        goff[gw * 2 + 1] = group_slot_offset(w, 1)

    # per-edge: slot position (core-local), dst slot, original edge id
    e_slot = goff[grp_of_edge] + slot_within          # core-local slot
    e_core = grp_of_edge // (2 * NW)
    e_dstloc = slot_of_node[g_dst]

    # per-slot gather index
    slot_idx = np.zeros((NCORES, SLOTS_CORE), dtype=np.int16)
    su = e_slot[is_slot]
    slot_idx[e_core[is_slot], su] = \
        (g_src[is_slot] - e_grp[e_order][is_slot] * SPLIT).astype(np.int16)

    inv_deg = (SCALE / np.maximum(deg, 1)).astype(np.float32)

    return dict(perm=perm, CE=CA, CO=CB, batches=batches,
                slot_idx=slot_idx, e_core=e_core, e_slot=e_slot,
                e_dstloc=e_dstloc, e_order=e_order,
                inv_deg=inv_deg, dst=dst,
                SLOTS_W=SLOTS_W, SLOTS_CORE=SLOTS_CORE)


def make_in_maps(prepd, x, edge_attr, w_q, w_k, w_v, w_o, b_o):
    """Build the per-core input dicts for the SPMD program."""
    from ml_dtypes import bfloat16

    CA, CB = prepd["CE"], prepd["CO"]
    perm = prepd["perm"]
    x = np.ascontiguousarray(x, dtype=np.float32)
    TOTCH = NW * (CA + CB)
    SLOTS_CORE = prepd["SLOTS_CORE"]

    xv = np.ascontiguousarray(x.astype(bfloat16))           # [N, D] bf16

    # per-edge coefficient a_e
    ew = 1.0 / (1.0 + np.exp(-np.asarray(edge_attr, np.float64).sum(axis=1)))
    av_edge = (ew.astype(np.float32) *
               prepd["inv_deg"][prepd["dst"]]).astype(np.float32)

    # head indicator matrices
    hmatT = np.zeros((P, H), dtype=bfloat16)
    for h in range(H):
        hmatT[h * DH:(h + 1) * DH, h] = 1
    hmat = np.ascontiguousarray(hmatT.T)                     # [H, P]
    bvec = np.asarray(b_o, np.float32).reshape(P, 1).copy()

    e_core = prepd["e_core"]
    e_slot = prepd["e_slot"]
    e_dstloc = prepd["e_dstloc"]
    e_av = av_edge[prepd["e_order"]]

    in_maps = []
    for core in range(NCORES):
        sidx = prepd["slot_idx"][core]
        S = SLOTS_CORE

        # gather indices, 16-wrapped PER CALL (A calls then B calls per batch)
        gidx_cols = []
        off = 0
        for wins in prepd["batches"]:
            nb = len(wins)
            for nc_ in _call_sizes(nb * CA) + _call_sizes(nb * CB):
                cnt = nc_ * P
                blk = sidx[off:off + cnt].reshape(cnt // 16, 16).T
                gidx_cols.append(np.tile(blk, (8, 1)))
                off += cnt
        gidx = np.ascontiguousarray(np.concatenate(gidx_cols, axis=1))

        # host-built scatter mask: [P, TOTCH*P] bf16, chunk-major columns.
        # Deduped slots can carry several edges -> accumulate in fp32 first.
        sel = e_core == core
        es = e_slot[sel]
        mask32 = np.zeros((P, TOTCH * P), dtype=np.float32)
        np.add.at(mask32, (es % P, (es // P) * P + e_dstloc[sel]), e_av[sel])
        mask = mask32.astype(bfloat16)
        del mask32

        # transposed node features for this core's windows (zeros for pads)
        nodes = perm[core * NW * P:(core + 1) * NW * P]
        xw = np.where(nodes[:, None] >= 0, x[np.maximum(nodes, 0)], 0.0)
        xwT = np.ascontiguousarray(xw.astype(bfloat16).T)    # [D, NW*P]

        in_maps.append(dict(
            xv=xv, xwT=xwT, gidx=gidx, mask=mask,
            wq=np.ascontiguousarray(np.asarray(w_q, np.float32).astype(bfloat16)),
            wk=np.ascontiguousarray(np.asarray(w_k, np.float32).astype(bfloat16)),
            wv=np.ascontiguousarray(np.asarray(w_v, np.float32).astype(bfloat16)),
            wo=np.ascontiguousarray(np.asarray(w_o, np.float32).astype(bfloat16)),
            hmatT=hmatT, hmat=hmat, bvec=bvec,
        ))
    return in_maps


# ========================== device program =================================

def build_program(CA, CB, batches):
    import concourse.bass as bass
    import concourse.mybir as mybir
    from concourse import bacc
    from concourse.tile import TileContext

    f32 = mybir.dt.float32
    bf16 = mybir.dt.bfloat16
    TOTCH = NW * (CA + CB)
    SLOTS_CORE = TOTCH * P
    NCH = CA + CB

    nc = bacc.Bacc("TRN2", target_bir_lowering=False, debug=False,
                   num_devices=NCORES, num_swdge_queues=4,
                   dynamic_dma_scratch_size=49152)

    xv_d = nc.dram_tensor("xv", [N, D], bf16, kind="ExternalInput")
    xwT_d = nc.dram_tensor("xwT", [D, NW * P], bf16, kind="ExternalInput")
    gidx_d = nc.dram_tensor("gidx", [P, SLOTS_CORE // 16], mybir.dt.int16,
                            kind="ExternalInput")
    mask_d = nc.dram_tensor("mask", [P, SLOTS_CORE], bf16, kind="ExternalInput")
    wq_d = nc.dram_tensor("wq", [D, D], bf16, kind="ExternalInput")
    wk_d = nc.dram_tensor("wk", [D, D], bf16, kind="ExternalInput")
    wv_d = nc.dram_tensor("wv", [D, D], bf16, kind="ExternalInput")
    wo_d = nc.dram_tensor("wo", [D, D], bf16, kind="ExternalInput")
    hmatT_d = nc.dram_tensor("hmatT", [P, H], bf16, kind="ExternalInput")
    hmat_d = nc.dram_tensor("hmat", [H, P], bf16, kind="ExternalInput")
    bvec_d = nc.dram_tensor("bvec", [P, 1], f32, kind="ExternalInput")
    out_d = nc.dram_tensor("out", [D, NW * P], bf16, kind="ExternalOutput")

    with TileContext(nc) as tc:
        with tc.tile_pool(name="consts", bufs=1) as consts, \
             tc.tile_pool(name="gather", bufs=40) as gpool, \
             tc.tile_pool(name="mask", bufs=2) as mpool, \
             tc.tile_pool(name="xt", bufs=2) as xpool, \
             tc.tile_pool(name="work", bufs=3) as wpool, \
             tc.tile_pool(name="gps", bufs=2, space="PSUM") as gpsum_pool, \
             tc.tile_pool(name="eps", bufs=5, space="PSUM") as epsum_pool, \
             tc.tile_pool(name="sps", bufs=1, space="PSUM") as spsum_pool:

            wq = consts.tile([D, D], bf16, tag="wq")
            wk = consts.tile([D, D], bf16, tag="wk")
            wv = consts.tile([D, D], bf16, tag="wv")
            wo = consts.tile([D, D], bf16, tag="wo")
            hmatT = consts.tile([P, H], bf16, tag="hmatT")
            hmat = consts.tile([H, P], bf16, tag="hmat")
            bvec = consts.tile([P, 1], f32, tag="bvec")
            gidx = consts.tile([P, SLOTS_CORE // 16], mybir.dt.int16, tag="gidx")
            for t, dsrc in ((wq, wq_d), (wk, wk_d), (wv, wv_d), (wo, wo_d),
                            (hmatT, hmatT_d), (hmat, hmat_d), (bvec, bvec_d),
                            (gidx, gidx_d)):
                nc.sync.dma_start(t[:], dsrc[:])

            xv_a = xv_d[0:SPLIT, :]
            xv_b = xv_d[SPLIT:N, :]

            qctr = 0       # SWDGE queue cycle
            gidx_col = 0   # running gidx column
            nreg = {}      # hoisted num_idxs registers

            def gather_calls(base_ap, sizes):
                """One dma_gather per call size; returns the call tiles."""
                nonlocal qctr, gidx_col
                tiles = []
                for nch in sizes:
                    cnt = nch * P
                    t = gpool.tile([P, GMAX, D], bf16, tag="gc")
                    if cnt not in nreg:
                        nreg[cnt] = nc.gpsimd.to_reg(cnt)
                    nc.gpsimd.dma_gather(
                        t[:, 0:nch, :], base_ap,
                        gidx[:, gidx_col:gidx_col + cnt // 16],
                        cnt, nreg[cnt], D,
                        queue_num=qctr % 4)
                    qctr += 1
                    gidx_col += cnt // 16
                    tiles.append(t)
                return tiles

            colbase = 0
            for b, wins in enumerate(batches):
                nb = len(wins)
                ncols = nb * P
                w0 = wins[0]

                mask_sb = mpool.tile([P, nb * NCH * P], bf16, tag="mask")
                nc.sync.dma_start(
                    mask_sb[:], mask_d[:, colbase * P:(colbase + nb * NCH) * P])
                xt_sb = xpool.tile([P, ncols], bf16, tag="xt")
                nc.sync.dma_start(xt_sb[:], xwT_d[:, w0 * P:w0 * P + ncols])

                atiles = gather_calls(xv_a, _call_sizes(nb * CA))
                btiles = gather_calls(xv_b, _call_sizes(nb * CB))

                # G^T accumulation: one PSUM bank holds all nb windows
                gps = gpsum_pool.tile([P, ncols], f32, tag="gps")
                for i in range(nb):
                    for c in range(NCH):
                        if c < CA:
                            g = i * CA + c
                            blk = atiles[g // GMAX][:, g % GMAX, :]
                            mcol = g * P
                        else:
                            g = i * CB + (c - CA)
                            blk = btiles[g // GMAX][:, g % GMAX, :]
                            mcol = (nb * CA + g) * P
                        nc.tensor.matmul(gps[:, i * P:(i + 1) * P],
                                         blk, mask_sb[:, mcol:mcol + P],
                                         start=(c == 0), stop=(c == NCH - 1))

                # ---- batched epilogue (transposed; no PE transposes) ----
                k_ps = epsum_pool.tile([P, ncols], f32, tag="ep")
                nc.tensor.matmul(k_ps[:], wk[:], xt_sb[:], start=True, stop=True)
                v_ps = epsum_pool.tile([P, ncols], f32, tag="ep")
                nc.tensor.matmul(v_ps[:], wv[:], xt_sb[:], start=True, stop=True)

                gt_sb = wpool.tile([P, ncols], bf16, tag="gt_sb")
                nc.scalar.copy(gt_sb[:], gps[:])
                ghat_ps = epsum_pool.tile([P, ncols], f32, tag="ep")
                nc.tensor.matmul(ghat_ps[:], wq[:], gt_sb[:],
                                 start=True, stop=True)
                ghat_sb = wpool.tile([P, ncols], f32, tag="ghat_sb")
                nc.scalar.copy(ghat_sb[:], ghat_ps[:])

                kg_sb = wpool.tile([P, ncols], bf16, tag="kg_sb")
                nc.vector.tensor_tensor(kg_sb[:], k_ps[:], ghat_sb[:],
                                        op=mybir.AluOpType.mult)

                s_ps = spsum_pool.tile([H, ncols], f32, tag="sps")
                nc.tensor.matmul(s_ps[:], hmatT[:], kg_sb[:],
                                 start=True, stop=True)
                s_sb = wpool.tile([H, ncols], bf16, tag="s_sb")
                nc.scalar.copy(s_sb[:], s_ps[:])
                sexp_ps = epsum_pool.tile([P, ncols], f32, tag="ep")
                nc.tensor.matmul(sexp_ps[:], hmat[:], s_sb[:],
                                 start=True, stop=True)

                v_sb = wpool.tile([P, ncols], f32, tag="v_sb")
                nc.scalar.copy(v_sb[:], v_ps[:])
                vs_sb = wpool.tile([P, ncols], bf16, tag="vs_sb")
                nc.vector.tensor_tensor(vs_sb[:], sexp_ps[:], v_sb[:],
                                        op=mybir.AluOpType.mult)

                o_ps = epsum_pool.tile([P, ncols], f32, tag="ep")
                nc.tensor.matmul(o_ps[:], wo[:], vs_sb[:], start=True, stop=True)
                o_sb = wpool.tile([P, ncols], bf16, tag="o_sb")
                nc.scalar.add(o_sb[:], o_ps[:], bvec[:, 0:1])
                nc.sync.dma_start(out_d[:, w0 * P:w0 * P + ncols], o_sb[:])

                colbase += nb * NCH

    nc.compile()
    return nc


# ============================ entry point ==================================

_PROGRAM_CACHE = {}


def kernel(**inputs):
    from concourse.bass_utils import run_bass_kernel_spmd

    x = np.asarray(inputs["x"], dtype=np.float32)
    edge_index = np.asarray(inputs["edge_index"])
    edge_attr = np.asarray(inputs["edge_attr"], dtype=np.float32)

    prepd = preprocess(edge_index)
    in_maps = make_in_maps(prepd, x, edge_attr,
                           inputs["w_q"], inputs["w_k"], inputs["w_v"],
                           inputs["w_o"], inputs["b_o"])

    key = (prepd["CE"], prepd["CO"])
    if key not in _PROGRAM_CACHE:
        _PROGRAM_CACHE[key] = build_program(prepd["CE"], prepd["CO"],
                                            prepd["batches"])
    nc = _PROGRAM_CACHE[key]

    res = run_bass_kernel_spmd(nc, in_maps, core_ids=list(range(NCORES)))

    out = np.zeros((N, D), dtype=np.float32)
    perm = prepd["perm"]
    for core in range(NCORES):
        rows = np.asarray(res.results[core]["out"]).astype(np.float32).T
        nodes = perm[core * NW * P:(core + 1) * NW * P]
        valid = nodes >= 0
        out[nodes[valid]] = rows[valid]
    return out


# revision 8
# speedup vs baseline: 1.2144x; 1.2144x over previous
"""Multi-head graph attention kernel for Trainium2 (8 NeuronCores, SPMD).

Math (algebraically equivalent to the reference):
  ew_e   = sigmoid(sum(edge_attr[e]))
  a_e    = ew_e * SCALE / max(deg[dst_e], 1)
  Gx[n]  = sum_{e: dst=n} a_e * x[src_e]            (segment sum of gathered rows)
  G      = Gx @ w_q ;  K = x @ w_k ;  V = x @ w_v
  S[n,h] = sum_{d in head h} K[n,d] * G[n,d]
  out    = (V * repeat(S, 16)) @ w_o + b_o

Device-side design (v2 — transposed, bf16, host-built masks):
  * Everything on device is bf16 (fp32 PSUM accumulation), halving both the
    random-row gather traffic and the streamed operands.
  * Nodes are permuted into NCORES*NW windows of 128 slots; every edge lives
    with its destination's window.  Edges are grouped by src range
    (src < 32768 vs >= 32768, so dma_gather's int16 indices can address the
    bf16 row directly) and padded to CA/CB chunks of 128.
  * The per-edge scatter coefficients a_e are baked on the HOST into a dense
    bf16 "mask" M[e_pos, slot] per chunk (one nonzero per row), streamed
    sequentially via HWDGE.  This removes the per-chunk DVE one-hot build —
    the v1 bottleneck (Vector 94.6% busy from SBUF-port contention with the
    SWDGE gather descriptor rings).
  * All compute is done transposed (features on partitions): the chunk
    matmul accumulates G^T[d, slot] = blk^T @ mask with the gathered rows as
    the stationary operand, so the epilogue needs zero PE transposes:
        K^T = wk^T X^T,  V^T = wv^T X^T,  Ghat^T = wq^T G^T
        S = Hmat (K^T*Ghat^T)   (per-head partition-group reduce via PE)
        out^T = wo^T (V^T * Hmat^T S) + b
    The epilogue is batched over NBATCH=4 windows ([128, 512] tiles).
"""

import math
import numpy as np

# ---------------- problem constants (hardcoded per the task) ----------------
N = 50000
E = 800000
D = 128
H = 8
DH = 16
DE = 16
SCALE = 1.0 / math.sqrt(DH)
NCORES = 8
P = 128          # node slots per window / partition dim
NW = 49          # windows per core  (NCORES*NW*P = 50176 >= N)
NBATCH = 4       # windows per epilogue batch
GMAX = 8         # max chunks (1024 idx) per dma_gather call (ring capacity
                 # caps calls at ~65 descs/engine; 1024 idx amortizes best)
SPLIT = 32768    # src-range split so gather indices fit int16


def _call_sizes(C):
    """Split C chunks into dma_gather call sizes of at most GMAX chunks."""
    out = []
    while C > 0:
        out.append(min(GMAX, C))
        C -= out[-1]
    return out


# ======================= host-side preprocessing ===========================

def preprocess(edge_index):
    """Index-only preprocessing: node permutation, edge grouping, padding."""
    src = np.asarray(edge_index[0], dtype=np.int64)
    dst = np.asarray(edge_index[1], dtype=np.int64)

    deg = np.bincount(dst, minlength=N)

    # node -> (window, slot): snake-deal by degree for load balance
    nwin_total = NCORES * NW
    order = np.argsort(-deg, kind="stable")
    slot_of_node = np.empty(N, dtype=np.int64)
    win_of_node = np.empty(N, dtype=np.int64)
    for r in range((N + nwin_total - 1) // nwin_total):
        chunk = order[r * nwin_total:(r + 1) * nwin_total]
        wins = np.arange(len(chunk))
        if r % 2 == 1:
            wins = nwin_total - 1 - wins
        win_of_node[chunk] = wins
        slot_of_node[chunk] = r
    assert slot_of_node.max() < P

    perm = np.full(nwin_total * P, -1, dtype=np.int64)
    perm[win_of_node * P + slot_of_node] = np.arange(N)

    # edges -> (window, src-range group), sorted by src inside each group.
    # Duplicate srcs within a group are DEDUPED: they share one gather slot
    # and the mask row gets multiple nonzeros.
    e_win = win_of_node[dst]
    e_grp = (src >= SPLIT).astype(np.int64)
    e_key = e_win * 2 + e_grp
    e_order = np.lexsort((src, e_key))
    g_src = src[e_order]
    g_dst = dst[e_order]

    ngrp = nwin_total * 2
    counts = np.bincount(e_key[e_order], minlength=ngrp)
    grp_start = np.concatenate([[0], np.cumsum(counts)])

    # unique-src slot within each group
    new_grp = np.zeros(E, dtype=bool)
    new_grp[grp_start[:-1][counts > 0]] = True
    new_src = np.empty(E, dtype=bool)
    new_src[0] = True
    new_src[1:] = g_src[1:] != g_src[:-1]
    is_slot = new_grp | new_src
    slot_within = np.arange(E) - np.maximum.accumulate(
        np.where(new_grp, np.arange(E), -1))
    # slot_within counts edges; recompute as unique-count within group:
    cum_slots = np.cumsum(is_slot)
    grp_of_edge = np.repeat(np.arange(ngrp), counts)
    slots_before_grp = np.concatenate(
        [[0], cum_slots[grp_start[1:-1] - 1]]) if ngrp > 1 else np.array([0])
    slot_within = cum_slots - 1 - slots_before_grp[grp_of_edge]
    ucounts = np.bincount(grp_of_edge[is_slot], minlength=ngrp)

    CA = int(np.ceil(ucounts[0::2].max() / P))
    CB = int(np.ceil(ucounts[1::2].max() / P))

    batches = [list(range(b, min(b + NBATCH, NW))) for b in range(0, NW, NBATCH)]
    SLOTS_W = (CA + CB) * P
    SLOTS_CORE = NW * SLOTS_W

    # slot layout: per batch, [A-chunks of its windows][B-chunks of its windows]
    batch_base = {}
    base = 0
    for b, wins in enumerate(batches):
        batch_base[b] = base
        base += len(wins) * SLOTS_W

    def group_slot_offset(w, grp):
        b, i = w // NBATCH, w % NBATCH
        nb = len(batches[b])
        if grp == 0:
            return batch_base[b] + i * CA * P
        return batch_base[b] + nb * CA * P + i * CB * P

    goff = np.zeros(ngrp, dtype=np.int64)
    for gw in range(nwin_total):
        w = gw % NW
        goff[gw * 2] = group_slot_offset(w, 0)
        goff[gw * 2 + 1] = group_slot_offset(w, 1)

    # per-edge: slot position (core-local), dst slot, original edge id
    e_slot = goff[grp_of_edge] + slot_within          # core-local slot
    e_core = grp_of_edge // (2 * NW)
    e_dstloc = slot_of_node[g_dst]

    # per-slot gather index
    slot_idx = np.zeros((NCORES, SLOTS_CORE), dtype=np.int16)
    su = e_slot[is_slot]
    slot_idx[e_core[is_slot], su] = \
        (g_src[is_slot] - e_grp[e_order][is_slot] * SPLIT).astype(np.int16)

    inv_deg = (SCALE / np.maximum(deg, 1)).astype(np.float32)

    return dict(perm=perm, CE=CA, CO=CB, batches=batches,
                slot_idx=slot_idx, e_core=e_core, e_slot=e_slot,
                e_dstloc=e_dstloc, e_order=e_order,
                inv_deg=inv_deg, dst=dst,
                SLOTS_W=SLOTS_W, SLOTS_CORE=SLOTS_CORE)


def make_in_maps(prepd, x, edge_attr, w_q, w_k, w_v, w_o, b_o):
    """Build the per-core input dicts for the SPMD program."""
    from ml_dtypes import bfloat16

    CA, CB = prepd["CE"], prepd["CO"]
    perm = prepd["perm"]
    x = np.ascontiguousarray(x, dtype=np.float32)
    TOTCH = NW * (CA + CB)
    SLOTS_CORE = prepd["SLOTS_CORE"]

    xv = np.ascontiguousarray(x.astype(bfloat16))           # [N, D] bf16

    # per-edge coefficient a_e
    ew = 1.0 / (1.0 + np.exp(-np.asarray(edge_attr, np.float64).sum(axis=1)))
    av_edge = (ew.astype(np.float32) *
               prepd["inv_deg"][prepd["dst"]]).astype(np.float32)

    # head indicator matrices
    hmatT = np.zeros((P, H), dtype=bfloat16)
    for h in range(H):
        hmatT[h * DH:(h + 1) * DH, h] = 1
    hmat = np.ascontiguousarray(hmatT.T)                     # [H, P]
    bvec = np.asarray(b_o, np.float32).reshape(P, 1).copy()

    e_core = prepd["e_core"]
    e_slot = prepd["e_slot"]
    e_dstloc = prepd["e_dstloc"]
    e_av = av_edge[prepd["e_order"]]

    in_maps = []
    for core in range(NCORES):
        sidx = prepd["slot_idx"][core]
        S = SLOTS_CORE

        # gather indices, 16-wrapped PER CALL (A calls then B calls per batch)
        gidx_cols = []
        off = 0
        for wins in prepd["batches"]:
            nb = len(wins)
            for nc_ in _call_sizes(nb * CA) + _call_sizes(nb * CB):
                cnt = nc_ * P
                blk = sidx[off:off + cnt].reshape(cnt // 16, 16).T
                gidx_cols.append(np.tile(blk, (8, 1)))
                off += cnt
        gidx = np.ascontiguousarray(np.concatenate(gidx_cols, axis=1))

        # host-built scatter mask: [P, TOTCH*P] bf16, chunk-major columns.
        # Deduped slots can carry several edges -> accumulate in fp32 first.
        sel = e_core == core
        es = e_slot[sel]
        mask32 = np.zeros((P, TOTCH * P), dtype=np.float32)
        np.add.at(mask32, (es % P, (es // P) * P + e_dstloc[sel]), e_av[sel])
        mask = mask32.astype(bfloat16)
        del mask32

        # transposed node features for this core's windows (zeros for pads)
        nodes = perm[core * NW * P:(core + 1) * NW * P]
        xw = np.where(nodes[:, None] >= 0, x[np.maximum(nodes, 0)], 0.0)
        xwT = np.ascontiguousarray(xw.astype(bfloat16).T)    # [D, NW*P]

        in_maps.append(dict(
            xv=xv, xwT=xwT, gidx=gidx, mask=mask,
            wq=np.ascontiguousarray(np.asarray(w_q, np.float32).astype(bfloat16)),
            wk=np.ascontiguousarray(np.asarray(w_k, np.float32).astype(bfloat16)),
            wv=np.ascontiguousarray(np.asarray(w_v, np.float32).astype(bfloat16)),
            wo=np.ascontiguousarray(np.asarray(w_o, np.float32).astype(bfloat16)),
            hmatT=hmatT, hmat=hmat, bvec=bvec,
        ))
    return in_maps


# ========================== device program =================================

def build_program(CA, CB, batches):
    import concourse.bass as bass
    import concourse.mybir as mybir
    from concourse import bacc
    from concourse.tile import TileContext

    f32 = mybir.dt.float32
    bf16 = mybir.dt.bfloat16
    TOTCH = NW * (CA + CB)
    SLOTS_CORE = TOTCH * P
    NCH = CA + CB

    nc = bacc.Bacc("TRN2", target_bir_lowering=False, debug=False,
                   num_devices=NCORES, num_swdge_queues=4,
                   dynamic_dma_scratch_size=49152)

    xv_d = nc.dram_tensor("xv", [N, D], bf16, kind="ExternalInput")
    xwT_d = nc.dram_tensor("xwT", [D, NW * P], bf16, kind="ExternalInput")
    gidx_d = nc.dram_tensor("gidx", [P, SLOTS_CORE // 16], mybir.dt.int16,
                            kind="ExternalInput")
    mask_d = nc.dram_tensor("mask", [P, SLOTS_CORE], bf16, kind="ExternalInput")
    wq_d = nc.dram_tensor("wq", [D, D], bf16, kind="ExternalInput")
    wk_d = nc.dram_tensor("wk", [D, D], bf16, kind="ExternalInput")
    wv_d = nc.dram_tensor("wv", [D, D], bf16, kind="ExternalInput")
    wo_d = nc.dram_tensor("wo", [D, D], bf16, kind="ExternalInput")
    hmatT_d = nc.dram_tensor("hmatT", [P, H], bf16, kind="ExternalInput")
    hmat_d = nc.dram_tensor("hmat", [H, P], bf16, kind="ExternalInput")
    bvec_d = nc.dram_tensor("bvec", [P, 1], f32, kind="ExternalInput")
    out_d = nc.dram_tensor("out", [D, NW * P], bf16, kind="ExternalOutput")

    with TileContext(nc) as tc:
        with tc.tile_pool(name="consts", bufs=1) as consts, \
             tc.tile_pool(name="gather", bufs=20) as gpool, \
             tc.tile_pool(name="mask", bufs=2) as mpool, \
             tc.tile_pool(name="xt", bufs=2) as xpool, \
             tc.tile_pool(name="work", bufs=3) as wpool, \
             tc.tile_pool(name="gps", bufs=2, space="PSUM") as gpsum_pool, \
             tc.tile_pool(name="eps", bufs=5, space="PSUM") as epsum_pool, \
             tc.tile_pool(name="sps", bufs=1, space="PSUM") as spsum_pool:

            wq = consts.tile([D, D], bf16, tag="wq")
            wk = consts.tile([D, D], bf16, tag="wk")
            wv = consts.tile([D, D], bf16, tag="wv")
            wo = consts.tile([D, D], bf16, tag="wo")
            hmatT = consts.tile([P, H], bf16, tag="hmatT")
            hmat = consts.tile([H, P], bf16, tag="hmat")
            bvec = consts.tile([P, 1], f32, tag="bvec")
            gidx = consts.tile([P, SLOTS_CORE // 16], mybir.dt.int16, tag="gidx")
            for t, dsrc in ((wq, wq_d), (wk, wk_d), (wv, wv_d), (wo, wo_d),
                            (hmatT, hmatT_d), (hmat, hmat_d), (bvec, bvec_d),
                            (gidx, gidx_d)):
                nc.sync.dma_start(t[:], dsrc[:])

            xv_a = xv_d[0:SPLIT, :]
            xv_b = xv_d[SPLIT:N, :]

            qctr = 0       # SWDGE queue cycle
            gidx_col = 0   # running gidx column
            nreg = {}      # hoisted num_idxs registers

            def gather_calls(base_ap, sizes):
                """One dma_gather per call size; returns the call tiles."""
                nonlocal qctr, gidx_col
                tiles = []
                for nch in sizes:
                    cnt = nch * P
                    t = gpool.tile([P, GMAX, D], bf16, tag="gc")
                    if cnt not in nreg:
                        nreg[cnt] = nc.gpsimd.to_reg(cnt)
                    nc.gpsimd.dma_gather(
                        t[:, 0:nch, :], base_ap,
                        gidx[:, gidx_col:gidx_col + cnt // 16],
                        cnt, nreg[cnt], D,
                        queue_num=qctr % 4)
                    qctr += 1
                    gidx_col += cnt // 16
                    tiles.append(t)
                return tiles

            colbase = 0
            for b, wins in enumerate(batches):
                nb = len(wins)
                ncols = nb * P
                w0 = wins[0]

                mask_sb = mpool.tile([P, nb * NCH * P], bf16, tag="mask")
                nc.sync.dma_start(
                    mask_sb[:], mask_d[:, colbase * P:(colbase + nb * NCH) * P])
                xt_sb = xpool.tile([P, ncols], bf16, tag="xt")
                nc.sync.dma_start(xt_sb[:], xwT_d[:, w0 * P:w0 * P + ncols])

                atiles = gather_calls(xv_a, _call_sizes(nb * CA))
                btiles = gather_calls(xv_b, _call_sizes(nb * CB))

                # G^T accumulation: one PSUM bank holds all nb windows
                gps = gpsum_pool.tile([P, ncols], f32, tag="gps")
                for i in range(nb):
                    for c in range(NCH):
                        if c < CA:
                            g = i * CA + c
                            blk = atiles[g // GMAX][:, g % GMAX, :]
                            mcol = g * P
                        else:
                            g = i * CB + (c - CA)
                            blk = btiles[g // GMAX][:, g % GMAX, :]
                            mcol = (nb * CA + g) * P
                        nc.tensor.matmul(gps[:, i * P:(i + 1) * P],
                                         blk, mask_sb[:, mcol:mcol + P],
                                         start=(c == 0), stop=(c == NCH - 1))

                # ---- batched epilogue (transposed; no PE transposes) ----
                k_ps = epsum_pool.tile([P, ncols], f32, tag="ep")
                nc.tensor.matmul(k_ps[:], wk[:], xt_sb[:], start=True, stop=True)
                v_ps = epsum_pool.tile([P, ncols], f32, tag="ep")
                nc.tensor.matmul(v_ps[:], wv[:], xt_sb[:], start=True, stop=True)

                gt_sb = wpool.tile([P, ncols], bf16, tag="gt_sb")
                nc.scalar.copy(gt_sb[:], gps[:])
                ghat_ps = epsum_pool.tile([P, ncols], f32, tag="ep")
                nc.tensor.matmul(ghat_ps[:], wq[:], gt_sb[:],
                                 start=True, stop=True)
                ghat_sb = wpool.tile([P, ncols], f32, tag="ghat_sb")
                nc.scalar.copy(ghat_sb[:], ghat_ps[:])

                kg_sb = wpool.tile([P, ncols], bf16, tag="kg_sb")
                nc.vector.tensor_tensor(kg_sb[:], k_ps[:], ghat_sb[:],
                                        op=mybir.AluOpType.mult)

                s_ps = spsum_pool.tile([H, ncols], f32, tag="sps")
                nc.tensor.matmul(s_ps[:], hmatT[:], kg_sb[:],
                                 start=True, stop=True)
                s_sb = wpool.tile([H, ncols], bf16, tag="s_sb")
                nc.scalar.copy(s_sb[:], s_ps[:])
                sexp_ps = epsum_pool.tile([P, ncols], f32, tag="ep")
                nc.tensor.matmul(sexp_ps[:], hmat[:], s_sb[:],
                                 start=True, stop=True)

                v_sb = wpool.tile([P, ncols], f32, tag="v_sb")
                nc.scalar.copy(v_sb[:], v_ps[:])
                vs_sb = wpool.tile([P, ncols], bf16, tag="vs_sb")
                nc.vector.tensor_tensor(vs_sb[:], sexp_ps[:], v_sb[:],
                                        op=mybir.AluOpType.mult)

                o_ps = epsum_pool.tile([P, ncols], f32, tag="ep")
                nc.tensor.matmul(o_ps[:], wo[:], vs_sb[:], start=True, stop=True)
                o_sb = wpool.tile([P, ncols], bf16, tag="o_sb")
                nc.scalar.add(o_sb[:], o_ps[:], bvec[:, 0:1])
                nc.sync.dma_start(out_d[:, w0 * P:w0 * P + ncols], o_sb[:])

                colbase += nb * NCH

    nc.compile()
    return nc


# ============================ entry point ==================================

_PROGRAM_CACHE = {}


def kernel(**inputs):
    from concourse.bass_utils import run_bass_kernel_spmd

    x = np.asarray(inputs["x"], dtype=np.float32)
    edge_index = np.asarray(inputs["edge_index"])
    edge_attr = np.asarray(inputs["edge_attr"], dtype=np.float32)

    prepd = preprocess(edge_index)
    in_maps = make_in_maps(prepd, x, edge_attr,
                           inputs["w_q"], inputs["w_k"], inputs["w_v"],
                           inputs["w_o"], inputs["b_o"])

    key = (prepd["CE"], prepd["CO"])
    if key not in _PROGRAM_CACHE:
        _PROGRAM_CACHE[key] = build_program(prepd["CE"], prepd["CO"],
                                            prepd["batches"])
    nc = _PROGRAM_CACHE[key]

    res = run_bass_kernel_spmd(nc, in_maps, core_ids=list(range(NCORES)))

    out = np.zeros((N, D), dtype=np.float32)
    perm = prepd["perm"]
    for core in range(NCORES):
        rows = np.asarray(res.results[core]["out"]).astype(np.float32).T
        nodes = perm[core * NW * P:(core + 1) * NW * P]
        valid = nodes >= 0
        out[nodes[valid]] = rows[valid]
    return out
